# revision 50
# baseline (speedup 1.0000x reference)
"""EntityBoundaryPredictor Bass kernel for 8 trn2 NeuronCores.

Reference computation (B=4, E=16, T=1024, H=1024, fp32):
    t   = token_embedding @ Wt + bt                       # [B,T,H]
    e   = entity_embedding @ We + be                      # [B,E,H]
    cls = einsum('beth,h->bet', relu(t[:,None]+e[:,:,None]), Wp) + bp
    cls = where(token_mask, cls, -1e4); p = sigmoid(cls)  # returns (cls, p)

Sharding: data-parallel over (b, token-half): core s -> b = s//2,
tokens [th*512,(th+1)*512) with th = s%2.  Weights replicated.

Host does everything small or output-elementwise: the entity projection
e' = ent@We + be + bt, bias folding, token compaction (only unmasked
tokens ship, padded to a 32 bucket), the output scatter (masked slots
get exact -1e4/0), +bp, and p = sigmoid(cls).

Device, per core (h on SBUF partitions for stage 1):
  stage 1  k-chunk loop: t'(k) = Wt[k]^T @ tok  (8 [128,TK] matmuls
           accumulating in PSUM); ACT casts t' PSUM -> SBUF f16 (the only
           cheap PSUM reader -- Pool can't touch PSUM, DVE loses its 4x
           mode on fp32).
  stage 2  m(k,e) = relu(t'(k) + e'(k,e)) tiles [128,TK] f16 built by
           DVE (4x perf mode, ~10/chunk) / Pool (4-5/chunk) / ACT
           (2/chunk); then per (token-tile tau, entity e) ONE PE matmul
           with the m SLICE as the STATIONARY operand and the Wp k-column
           as the 1-wide moving operand:
               cls_ps[:, e*NT+tau] += m[:, tau*128:...]^T @ wp[:, k]
           Output free size is 1, so these 512 matmuls are ~free on the
           PE; all 64 accumulator chains live in ONE PSUM bank (a single
           start=True zeroes the whole bank row for every chain).
  finalize per entity group of 4 as its chains stop: cast [P, 4*NT]
           PSUM -> SBUF f16 (ACT; last group on DVE behind its own final
           m-build), out DMAs fanned over SP/Pool queues.

The m-build is the critical resource: 128 tiles split so DVE/ACT/Pool all
finish together (185/585/400 ns per tile).  The PE (projection + free
matvecs) and all DMA traffic hide underneath.
"""

import os

import numpy as np

import bass_rust as _bass_rust
import concourse.bacc as bacc
import concourse.mybir as mybir
from concourse.hw_specs import get_activation_tables
from concourse.tile import TileContext
from concourse.bass_utils import run_bass_kernel_spmd

B, E, T, H = 4, 16, 1024, 1024
P = 128
NCORES = 8
TS = T // 2          # tokens per core (pre-compaction)
HC = H // P          # h-chunks (contraction)
KC = H // P          # k-chunks (projected feature dim; == h of stage 2)
NEG = -10000.0

F32 = mybir.dt.float32
F16 = mybir.dt.float16

CFG = {
    # engine assignment pattern for the 16 m-tiles of each k-chunk:
    # counts for (DVE, ACT, Pool); remainder goes to DVE
    # per-round engine split: digit strings, one digit per k-chunk
    "act_pat": os.environ.get("K_ACT_PAT", "22222122"),
    "pool_pat": os.environ.get("K_POOL_PAT", "55444444"),
    # PE p-state warmup matmuls (64-col dummies) before real work
    "warm_n": int(os.environ.get("K_WARM_N", "36")),
    # PE emission lag (chunks) of stage2 behind the projection
    "lag": int(os.environ.get("K_LAG", "2")),
    "psa_bufs": int(os.environ.get("K_PSA_BUFS", "2")),
    "tp_bufs": int(os.environ.get("K_TP_BUFS", "2")),
    "m_bufs": int(os.environ.get("K_M_BUFS", "48")),
    # token column where the last per-round tile is split DVE/Pool
    # (0 = no split, Pool gets nothing)
    "split_col": int(os.environ.get("K_SPLIT_COL", "0")),
    # t'-copy pairing scheme: none | 066 | tail | all
    "pairing": os.environ.get("K_PAIRING", "none"),
}

LAST_RESULTS = None  # BassKernelResults of the most recent run (for test.py)
_BUILT = None        # (cfg_key, nc)


def build(cfg=None, tk=TS):
    cfg = cfg or CFG
    TK = tk
    NT = (TK + P - 1) // P       # token tiles per core
    nc = bacc.Bacc("TRN2", target_bir_lowering=False, debug=False)

    # All ACT funcs used here (Identity/Relu) live in the sigmoid_and_others
    # set; blank the other sets (ids preserved) so one table load suffices.
    def _one_table_set():
        if not any(
            isinstance(i, mybir.InstActivation)
            for b in nc.main_func.blocks
            for i in b.instructions
        ):
            return
        tables = [
            (n, (f if n == "sigmoid_and_others" else set()))
            for n, f in get_activation_tables(nc.m.arch).items()
        ]
        _bass_rust.insert_act_table_loads(nc, tables)

    nc.insert_act_table_loads = _one_table_set

    tok = nc.declare_dram_parameter("tok", [P, HC, TK], F16, isOutput=False)
    wt = nc.declare_dram_parameter("wt", [P, KC, HC, P], F16, isOutput=False)
    # e' + be + bt precomputed on the host, [p, k, e] lane-major
    ep = nc.declare_dram_parameter("ep", [P, KC * E], F32, isOutput=False)
    # Wp columns, lane-major f16 (matmul dtype): wp[p, k] = Wp[k*128+p]
    wpd = nc.declare_dram_parameter("wp", [P, KC], F16, isOutput=False)
    # f16 cls output: out[p, tau*16+e] = cls[entity e, token tau*128+p]
    # (kept tokens only; host scatters back, adds bp, computes sigmoid)
    out = nc.declare_dram_parameter("out", [P, NT * E], F16, isOutput=True)

    Act = mybir.ActivationFunctionType
    Alu = mybir.AluOpType

    act_pat = [int(c) for c in cfg["act_pat"]]
    pool_pat = [int(c) for c in cfg["pool_pat"]]
    warm_n = cfg["warm_n"]
    lag = cfg["lag"]

    with TileContext(nc) as tc:
        with (
            tc.tile_pool(name="const", bufs=1) as cpool,
            tc.tile_pool(name="mt", bufs=cfg["m_bufs"]) as mpool,
            tc.tile_pool(name="psA", bufs=cfg["psa_bufs"], space="PSUM") as psA,
            tc.tile_pool(name="psC", bufs=1, space="PSUM") as psC,
            tc.tile_pool(name="psW", bufs=1, space="PSUM") as psW,
        ):
            # ---- PE p-state warmup: starts the ramp clock at t~0 -----------
            if warm_n > 0:
                warm = cpool.tile([P, 64], F16, tag="warm")
                nc.gpsimd.memset(warm[:, :], 0.0)
                wps = psW.tile([64, 64], F32, tag="ps_warm")
                for w in range(warm_n):
                    nc.tensor.matmul(
                        wps[:, :], lhsT=warm[:, 0:64], rhs=warm[:, :],
                        start=(w == 0), stop=(w == warm_n - 1),
                    )

            # ---- SBUF tiles ------------------------------------------------
            tok_sb = cpool.tile([P, HC, TK], F16, tag="tok")
            ep_sb = cpool.tile([P, KC, E], F32, tag="ep")    # e'+be+bt [k, e]
            ep_sb_flat = ep_sb[:, :, :].rearrange("p k e -> p (k e)")
            wt_sb = cpool.tile([P, KC, HC, P], F16, tag="wt")
            wp16 = cpool.tile([P, KC], F16, tag="wp16")

            # ---- input DMAs: first wave, fanned across issue queues --------
            # NOTHING on the ACT queue: ACT's engine time is all needed for
            # the t'-copies + its m-build share (DMA cost occupies the
            # issuing engine in the cost model)
            # tok chunk DMAs land in the order proj(0) consumes them; the PE
            # streams matmul hc as soon as chunk hc + wt0 arrive.  ACT joins
            # late (behind LoadActFuncSet) so it carries only the last chunk.
            nc.sync.dma_start(out=wt_sb[:, 0], in_=wt[:, 0])
            nc.gpsimd.dma_start(out=tok_sb[:, 0:1, :], in_=tok[:, 0:1, :])
            nc.gpsimd.dma_start(out=tok_sb[:, 1:2, :], in_=tok[:, 1:2, :])
            nc.sync.dma_start(out=tok_sb[:, 2:3, :], in_=tok[:, 2:3, :])
            nc.sync.dma_start(out=tok_sb[:, 3:4, :], in_=tok[:, 3:4, :])
            nc.gpsimd.dma_start(out=tok_sb[:, 4:5, :], in_=tok[:, 4:5, :])
            nc.gpsimd.dma_start(out=tok_sb[:, 5:6, :], in_=tok[:, 5:6, :])
            nc.sync.dma_start(out=tok_sb[:, 6:7, :], in_=tok[:, 6:7, :])
            # the last tok chunk rides ACT's dead time behind LoadActFuncSet
            nc.scalar.dma_start(out=tok_sb[:, 7:8, :], in_=tok[:, 7:8, :])
            nc.sync.dma_start(out=ep_sb_flat[:, :], in_=ep[:, :])
            nc.sync.dma_start(out=wp16[:, :], in_=wpd[:, :])
            nc.sync.dma_start(out=wt_sb[:, 1], in_=wt[:, 1])

            # t' f16 staging (one chunk at a time, double buffered)
            tpool_cm = tc.tile_pool(name="tp", bufs=cfg["tp_bufs"])
            tpool = tpool_cm.__enter__()

            # all 64 (tau, e) accumulator chains in ONE PSUM bank
            cls_ps = psC.tile([P, NT * E], F32, tag="cls")
            if TK % P != 0:
                # partitions >= TK-tau*P of the last tau's columns are never
                # written by matvecs; initialize so the finalize can read the
                # full tile (host ignores those rows)
                nc.vector.memset(cls_ps[:, :], 0.0)

            ps_list = [None] * KC    # live t' PSUM tiles per chunk
            tp_list = [None] * KC    # live t' SBUF f16 tiles per chunk

            out_sb = cpool.tile([P, NT * E], F16, tag="out_sb")
            out_qs = [nc.sync, nc.gpsimd, nc.gpsimd, nc.sync]

            def stage2(k):
                """m tiles + stationary matvecs for chunk k."""
                tp_sb = tp_list[k]    # an AP covering chunk k's t' in SBUF
                n_act = act_pat[k]
                npool_k = pool_pat[k]
                sc_col = cfg["split_col"]
                for e in range(E):
                    m = mpool.tile([P, TK], F16, tag="m")
                    sc = ep_sb[:, k, e : e + 1]
                    if e < n_act:
                        nc.scalar.activation(
                            m[:, :], tp_sb, Act.Relu, bias=sc,
                        )
                    elif e < n_act + npool_k:
                        nc.gpsimd.tensor_scalar(
                            out=m[:, :], in0=tp_sb,
                            scalar1=sc, scalar2=0.0,
                            op0=Alu.add, op1=Alu.max,
                        )
                    else:
                        nc.vector.tensor_scalar(
                            out=m[:, :], in0=tp_sb,
                            scalar1=sc, scalar2=0.0,
                            op0=Alu.add, op1=Alu.max,
                        )
                    for tau in range(NT):
                        t0 = tau * P
                        t1 = min(t0 + P, TK)
                        # ONE start for the whole bank: start=True zeroes the
                        # full 2KB bank row (ZERO_REGION) for every partition,
                        # so the first matvec's start covers all 64 chains --
                        # later chains' first writes land on pending-zero
                        # bytes and overwrite, then accumulate.
                        nc.tensor.matmul(
                            cls_ps[0 : t1 - t0, e * NT + tau : e * NT + tau + 1],
                            lhsT=m[:, t0:t1],
                            rhs=wp16[:, k : k + 1],
                            start=(k == 0 and e == 0 and tau == 0),
                            stop=(k == KC - 1),
                            skip_group_check=True,
                        )
                    # finalize each entity group of 4 the moment its last
                    # chain stops: cast to f16 on DVE (cheap, and free after
                    # its last m-build) + DMA out, fanned over queues
                    if k == KC - 1 and e % 4 == 3:
                        g = e // 4
                        c0, c1 = g * 4 * NT, (g + 1) * 4 * NT
                        if g == 3:
                            # the last group finalizes on DVE, right behind
                            # its own final m-build in the DVE FIFO
                            nc.vector.tensor_copy(out=out_sb[:, c0:c1],
                                                  in_=cls_ps[:, c0:c1])
                        else:
                            nc.scalar.activation(out_sb[:, c0:c1],
                                                 cls_ps[:, c0:c1],
                                                 Act.Identity)
                        out_qs[g].dma_start(out=out[:, c0:c1],
                                            in_=out_sb[:, c0:c1])

            # pairing scheme for t' chunks: chunks in a pair share one
            # 2-bank-aligned PSUM tile and ONE ACT copy (985ns vs 2x585).
            # Chunks 0 and 7 stay single so round 0 starts early and the
            # last round isn't delayed.  pair_of[k] = (partner, slot)
            pairing = cfg["pairing"]
            if pairing == "066":
                pair_start = {1: 2, 3: 4, 5: 6}
            elif pairing == "all":
                pair_start = {0: 1, 2: 3, 4: 5, 6: 7}
            elif pairing == "tail":
                pair_start = {2: 3, 4: 5, 6: 7}
            else:
                pair_start = {}
            pair_slot = {}
            for a, b in pair_start.items():
                pair_slot[a] = (a, 0)
                pair_slot[b] = (a, 1)
            PW = 512            # bank-aligned f32 pitch per chunk in a pair
            pair_ps = {}        # first-chunk -> psum pair tile
            pair_tp = {}        # first-chunk -> sbuf pair tile

            for k in range(KC + lag):
                if k < KC:
                    # token projection chunk k
                    if k in pair_slot:
                        a, slot = pair_slot[k]
                        if slot == 0:
                            pair_ps[a] = psA.tile([P, 2, PW], F32,
                                                  tag="ps_p", name=f"psp{a}")
                            pair_tp[a] = tpool.tile([P, 2, TK], F16,
                                                    tag="tp_p", name=f"tpp{a}")
                        ps = pair_ps[a][:, slot, 0:TK]
                        tp_sb = pair_tp[a][:, slot, :]
                    else:
                        ps_tile = psA.tile([P, TK], F32, tag="ps_t",
                                           name=f"pst{k}")
                        tp_tile = tpool.tile([P, TK], F16, tag="tp",
                                             name=f"tpt{k}")
                        ps = ps_tile[:, :]
                        tp_sb = tp_tile[:, :]
                    ps_list[k] = ps
                    tp_list[k] = tp_sb
                    for hc in range(HC):
                        nc.tensor.matmul(
                            ps,
                            lhsT=wt_sb[:, k, hc, :],
                            rhs=tok_sb[:, hc, :],
                            start=(hc == 0),
                            stop=(hc == HC - 1),
                        )
                    # t' PSUM -> SBUF f16 on ACT (Pool cannot access PSUM;
                    # DVE's 4x mode needs the f16 SBUF source); pairs copy
                    # both chunks in one op after the second lands
                    if k not in pair_slot:
                        nc.scalar.activation(tp_sb, ps, Act.Identity)
                    elif pair_slot[k][1] == 1:
                        a = pair_slot[k][0]
                        nc.scalar.activation(
                            pair_tp[a][:, :, :],
                            pair_ps[a][:, :, 0:TK],
                            Act.Identity,
                        )
                    # prefetch weight chunk k+2 (first wave covered 0 and 1)
                    # on SP -- keep ACT's engine time free
                    if k + 2 < KC:
                        nc.sync.dma_start(out=wt_sb[:, k + 2], in_=wt[:, k + 2])
                if k >= lag:
                    stage2(k - lag)

            tpool_cm.__exit__(None, None, None)

    nc.compile()
    return nc


def shard_inputs(token_embedding, entity_embedding, token_mask, Wt, bt, We, be,
                 Wp, bp):
    """Prepare per-core inputs.  The token dimension is COMPACTED: only
    unmasked tokens are shipped (the device never computes the masked
    columns -- the host writes their exact -1e4 / 0 values during the
    scatter).  Returns (in_maps, tk, keep)."""
    f16 = np.float16
    f32 = np.float32

    # weights shared (replicated) across all cores
    # wtR[p, kc, hc, j] = Wt[hc*128+p, kc*128+j]
    wtR = np.ascontiguousarray(
        Wt.astype(f16).reshape(HC, P, KC, P).transpose(1, 2, 0, 3))
    wpR = np.ascontiguousarray(Wp.astype(f32).reshape(KC, P).T.astype(f16))
    # host-side entity projection: e' = ent @ We + be + bt  [B, E, H]
    e2 = (entity_embedding.reshape(B * E, H).astype(f32) @ We.astype(f32)
          + (be.astype(f32) + bt.astype(f32))[None, :]).reshape(B, E, H)

    keep = []
    for s in range(NCORES):
        b, th = divmod(s, 2)
        tsl = slice(th * TS, (th + 1) * TS)
        keep.append(np.flatnonzero(np.asarray(token_mask[b, tsl])))
    # pad the kept-token count to a bucket (multiple of 32, at least 32)
    tk = max(32, -(-max(len(kp) for kp in keep) // 32) * 32)

    in_maps = []
    for s in range(NCORES):
        b, th = divmod(s, 2)
        tsl = slice(th * TS, (th + 1) * TS)
        kp = keep[s]
        sl = token_embedding[b, tsl, :][kp, :].astype(f16)       # [nk, H]
        if len(kp) < tk:
            sl = np.concatenate(
                [sl, np.zeros((tk - len(kp), H), f16)], axis=0)
        tokc = np.ascontiguousarray(
            sl.T.reshape(HC, P, tk).transpose(1, 0, 2))
        # ep[p, k, e] = e2[b, e, k*128+p] -> flattened [P, KC*E]
        epc = np.ascontiguousarray(
            e2[b].T.reshape(KC, P, E).transpose(1, 0, 2).reshape(P, KC * E))
        in_maps.append({
            "tok": tokc, "wt": wtR, "ep": epc, "wp": wpR,
        })
    return in_maps, tk, keep


def kernel(token_embedding, entity_embedding, token_mask, Wt, bt, We, be, Wp, bp):
    global LAST_RESULTS, _BUILT
    # accept jax or numpy inputs; all host prep runs in numpy
    token_embedding = np.asarray(token_embedding)
    entity_embedding = np.asarray(entity_embedding)
    token_mask = np.asarray(token_mask)
    Wt, bt = np.asarray(Wt), np.asarray(bt)
    We, be = np.asarray(We), np.asarray(be)
    Wp, bp = np.asarray(Wp), np.asarray(bp)
    in_maps, tk, keep = shard_inputs(token_embedding, entity_embedding,
                                     token_mask, Wt, bt, We, be, Wp, bp)
    cfg_key = (tuple(sorted(CFG.items())), tk)
    if _BUILT is None or _BUILT[0] != cfg_key:
        _BUILT = (cfg_key, build(CFG, tk=tk))
    nc = _BUILT[1]

    trace = os.environ.get("K_TRACE", "0") == "1"
    res = run_bass_kernel_spmd(nc, in_maps, core_ids=list(range(NCORES)),
                               trace=trace)
    LAST_RESULTS = res

    NT = (tk + P - 1) // P
    bpf = float(np.asarray(bp, np.float32).reshape(-1)[0])
    # scatter the kept columns back; masked slots get exact -1e4 / 0.
    cls = np.full((B, E, T), np.float32(NEG))
    p = np.zeros((B, E, T), np.float32)
    for s in range(NCORES):
        b, th = divmod(s, 2)
        kp = keep[s]
        o = res.results[s]["out"].astype(np.float32)     # [P, NT*E]
        # o[p, e*NT + tau] = cls[e, tau*128+p] (without bp)
        o = o.reshape(P, E, NT).transpose(1, 2, 0).reshape(E, NT * P)
        o = o[:, 0 : len(kp)] + bpf
        cls[b, :, th * TS + kp] = o.T
        p[b, :, th * TS + kp] = (1.0 / (1.0 + np.exp(-o))).T
    return cls, p
